# revision 2
# baseline (speedup 1.0000x reference)
"""GAT layer (nn_SPACY_GraphAttentionLayer) Trainium2 Bass kernel.

Data-parallel over batch: 8 graphs -> 8 NeuronCores, one graph per core.

Math (per graph):
  Wh = h @ W1, Wj = j @ W2, V = Wh + Wj
  Wh1_i = (Wh @ a1)_i,  Wh2_k = (Wh @ a2)_k
  z[i,k] = Wh1_i + Wh2_k
  logits = where(adj>0, leaky_relu(z, 0.2), -inf)
  A = softmax(logits, axis k);  out = elu(A @ V)

Factorization used on-chip:
  exp(lrelu(z)) = exp(0.8*relu(z)) * exp(0.2*Wh1_i) * exp(0.2*Wh2_k)
  The exp(0.2*Wh1_i) row factor cancels in the softmax.  With
  q = exp(0.8*relu(z)), p = adj * q, e_k = exp(0.2*Wh2_k):
     A[i,k] = p[i,k]*e_k / sum_k p[i,k]*e_k
     out_row = elu( (p_row @ (e*V)) / (p_row @ e) )
"""
import sys
import numpy as np

sys.path.insert(0, "/opt/trn_rl_repo")

B, N, F, O = 8, 2048, 256, 128
P = 128
NB = N // P  # 16 row/col chunks

_CACHE = {}


def _build_nc():
    import ml_dtypes
    from contextlib import ExitStack
    import concourse.bass as bass
    import concourse.tile as tile
    from concourse import bacc, mybir

    f32 = mybir.dt.float32
    bf16 = mybir.dt.bfloat16
    fp16 = mybir.dt.float16
    i32 = mybir.dt.int32
    Alu = mybir.AluOpType
    Act = mybir.ActivationFunctionType

    nc = bacc.Bacc()
    h_d = nc.dram_tensor("h", [N, F], f32, kind="ExternalInput")
    j_d = nc.dram_tensor("j", [N, F], f32, kind="ExternalInput")
    adj_d = nc.dram_tensor("adj", [N, N], i32, kind="ExternalInput")
    W1_d = nc.dram_tensor("W1", [F, O], f32, kind="ExternalInput")
    W2_d = nc.dram_tensor("W2", [F, O], f32, kind="ExternalInput")
    a_d = nc.dram_tensor("a", [2 * O, 1], f32, kind="ExternalInput")
    out_d = nc.dram_tensor("out", [N, O], f32, kind="ExternalOutput")

    identb_d = nc.inline_tensor(np.eye(P, dtype=ml_dtypes.bfloat16), name="identb")
    identh_d = nc.inline_tensor(np.eye(P, dtype=np.float16), name="identh")
    identf_d = nc.inline_tensor(np.eye(P, dtype=np.float32), name="identf")

    with tile.TileContext(nc) as tc, ExitStack() as ctx:
        cpool = ctx.enter_context(tc.tile_pool(name="cpool", bufs=1))
        wpool = ctx.enter_context(tc.tile_pool(name="wpool", bufs=1))
        bpool = ctx.enter_context(tc.tile_pool(name="bpool", bufs=1))
        apool = ctx.enter_context(tc.tile_pool(name="apool", bufs=4))
        qpool = ctx.enter_context(tc.tile_pool(name="qpool", bufs=2))
        spool = ctx.enter_context(tc.tile_pool(name="spool", bufs=2))
        psM = ctx.enter_context(tc.tile_pool(name="psM", bufs=2, space="PSUM"))
        psT = ctx.enter_context(tc.tile_pool(name="psT", bufs=3, space="PSUM"))
        psO = ctx.enter_context(tc.tile_pool(name="psO", bufs=2, space="PSUM"))

        # ---------------- Stage A: weights prep ----------------
        identb = cpool.tile([P, P], bf16, tag="identb")
        nc.sync.dma_start(identb[:], identb_d[:])
        identf = cpool.tile([P, P], f32, tag="identf")
        nc.sync.dma_start(identf[:], identf_d[:])
        identh = cpool.tile([P, P], fp16, tag="identh")
        nc.sync.dma_start(identh[:], identh_d[:])
        ones1 = cpool.tile([1, P], f32, tag="ones1")
        nc.vector.memset(ones1[:], 1.0)

        w1s = wpool.tile([P, 2, O], f32, tag="w1s")
        nc.sync.dma_start(w1s[:], W1_d.rearrange("(c p) o -> p c o", p=P))
        w2s = wpool.tile([P, 2, O], f32, tag="w2s")
        nc.sync.dma_start(w2s[:], W2_d.rearrange("(c p) o -> p c o", p=P))
        a12 = wpool.tile([P, 2], f32, tag="a12")
        nc.sync.dma_start(a12[:], a_d.rearrange("(c p) one -> p (c one)", p=P))

        w1b = wpool.tile([P, 2, O], fp16, tag="w1b")
        nc.vector.tensor_copy(w1b[:], w1s[:])
        w2b = wpool.tile([P, 2, O], fp16, tag="w2b")
        nc.vector.tensor_copy(w2b[:], w2s[:])
        a12b = wpool.tile([P, 2], fp16, tag="a12b")
        nc.vector.tensor_copy(a12b[:], a12[:])

        # W1^T chunks (bf16) then w1a | w2a = W1 @ [a1 | a2]
        w1t_ps = psM.tile([P, 2, P], fp16, tag="psM")
        for c in range(2):
            nc.tensor.transpose(w1t_ps[:, c, :], w1b[:, c, :], identh[:])
        w1t = wpool.tile([P, 2, P], fp16, tag="w1t")
        nc.vector.tensor_copy(w1t[:], w1t_ps[:])
        wab_ps = psM.tile([P, 2, 2], f32, tag="psM")
        for c in range(2):
            nc.tensor.matmul(wab_ps[:, c, :], w1t[:, c, :], a12b[:], start=True, stop=True)
        wab = wpool.tile([P, 2, 2], fp16, tag="wab")
        nc.vector.tensor_copy(wab[:], wab_ps[:])

        # rhs for projections: r1 = [W1 | w1a | w2a], r2z = [W2 | 0 | 0]
        r1 = wpool.tile([P, 2, 130], fp16, tag="r1")
        nc.vector.tensor_copy(r1[:, :, 0:128], w1b[:])
        nc.vector.tensor_copy(r1[:, :, 128:130], wab[:])
        r2z = wpool.tile([P, 2, 130], fp16, tag="r2z")
        nc.vector.memset(r2z[:], 0.0)
        nc.vector.tensor_copy(r2z[:, :, 0:128], w2b[:])

        # ---------------- Stage B: load h/j, transpose, project ----------------
        hf = bpool.tile([P, NB, F], f32, tag="hf")
        nc.sync.dma_start(hf[:], h_d.rearrange("(n p) f -> p n f", p=P))
        jf = bpool.tile([P, NB, F], f32, tag="jf")
        nc.sync.dma_start(jf[:], j_d.rearrange("(n p) f -> p n f", p=P))
        hb = bpool.tile([P, NB, F], fp16, tag="hb")
        nc.vector.tensor_copy(hb[:], hf[:])
        jb = bpool.tile([P, NB, F], fp16, tag="jb")
        nc.scalar.copy(jb[:], jf[:])

        hT0 = bpool.tile([P, NB, P], fp16, tag="hT0")
        hT1 = bpool.tile([P, NB, P], fp16, tag="hT1")
        jT0 = bpool.tile([P, NB, P], fp16, tag="jT0")
        jT1 = bpool.tile([P, NB, P], fp16, tag="jT1")
        for g in range(4):  # groups of 4 chunks
            tph = psT.tile([P, 8, P], fp16, tag="psT")
            tpj = psT.tile([P, 8, P], fp16, tag="psT")
            for i in range(4):
                c16 = 4 * g + i
                nc.tensor.transpose(tph[:, i, :], hb[:, c16, 0:128], identh[:])
                nc.tensor.transpose(tph[:, 4 + i, :], hb[:, c16, 128:256], identh[:])
                nc.tensor.transpose(tpj[:, i, :], jb[:, c16, 0:128], identh[:])
                nc.tensor.transpose(tpj[:, 4 + i, :], jb[:, c16, 128:256], identh[:])
            s = slice(4 * g, 4 * g + 4)
            nc.vector.tensor_copy(hT0[:, s, :], tph[:, 0:4, :])
            nc.vector.tensor_copy(hT1[:, s, :], tph[:, 4:8, :])
            nc.scalar.copy(jT0[:, s, :], tpj[:, 0:4, :])
            nc.scalar.copy(jT1[:, s, :], tpj[:, 4:8, :])

        wh1 = bpool.tile([P, NB], f32, tag="wh1")    # Wh1 column per row-block
        wh2g = bpool.tile([P, NB], f32, tag="wh2g")  # Wh2 column per col-chunk
        ecf = bpool.tile([P, NB], f32, tag="ecf")    # e = exp(0.2*Wh2) per chunk
        rhs_att = bpool.tile([P, NB, 129], fp16, tag="rhs_att")  # [e*V | e] per chunk

        for c16 in range(NB):
            psv = psO.tile([P, 130], f32, tag="psO")
            nc.tensor.matmul(psv[:], hT0[:, c16, :], r1[:, 0, :], start=True, stop=False)
            nc.tensor.matmul(psv[:], hT1[:, c16, :], r1[:, 1, :], start=False, stop=False)
            nc.tensor.matmul(psv[:], jT0[:, c16, :], r2z[:, 0, :], start=False, stop=False)
            nc.tensor.matmul(psv[:], jT1[:, c16, :], r2z[:, 1, :], start=False, stop=True)
            nc.vector.tensor_copy(wh1[:, c16 : c16 + 1], psv[:, 128:129])
            nc.vector.tensor_copy(wh2g[:, c16 : c16 + 1], psv[:, 129:130])
            nc.scalar.activation(ecf[:, c16 : c16 + 1], psv[:, 129:130], Act.Exp, scale=0.2)
            # V' = e_k * (Wh + Wj), bf16; plus e column at position 128
            nc.vector.tensor_scalar(
                rhs_att[:, c16, 0:128], psv[:, 0:128], ecf[:, c16 : c16 + 1], None, Alu.mult
            )
            nc.vector.tensor_copy(rhs_att[:, c16, 128:129], ecf[:, c16 : c16 + 1])

        # Wh2 broadcast tile [P, N] bf16: wh2g -> transpose -> flatten row -> ones outer-product
        t16ps = psM.tile([16, P], f32, tag="psM")
        nc.tensor.transpose(t16ps[:], wh2g[:], identf[:])
        t16s = bpool.tile([16, P], f32, tag="t16s")
        nc.vector.tensor_copy(t16s[:], t16ps[:])
        wh2row = bpool.tile([1, N], f32, tag="wh2row")
        nc.sync.dma_start(wh2row[:], t16s[:])  # SBUF->SBUF partition-major flatten
        wh2bc = bpool.tile([P, N], fp16, tag="wh2bc")
        for s4 in range(4):
            psbc = psM.tile([P, 512], f32, tag="psM")
            nc.tensor.matmul(
                psbc[:], ones1[:], wh2row[0:1, s4 * 512 : (s4 + 1) * 512], start=True, stop=True
            )
            nc.vector.tensor_copy(wh2bc[:, s4 * 512 : (s4 + 1) * 512], psbc[:])

        # ---------------- Stage C: attention row-blocks ----------------
        outbuf = bpool.tile([P, NB, O], f32, tag="outbuf")
        for rb in range(NB):
            adjb = apool.tile([P, N], bf16, tag="adjb")
            nc.gpsimd.dma_start(adjb[:], adj_d[rb * P : (rb + 1) * P, :])  # i32->bf16 cast

            r = qpool.tile([P, N], fp16, tag="r")
            nc.vector.tensor_scalar(
                r[:], wh2bc[:], wh1[:, rb : rb + 1], 0.0, Alu.add, Alu.max
            )
            q = qpool.tile([P, N], bf16, tag="q")
            nc.scalar.activation(q[:], r[:], Act.Exp, scale=0.8)
            p = qpool.tile([P, N], bf16, tag="p")
            nc.vector.tensor_tensor(p[:], q[:], adjb[:], Alu.mult)

            pt = qpool.tile([P, NB, P], bf16, tag="pt")
            for half in range(2):
                tps = psT.tile([P, 8, P], bf16, tag="psT")
                for c8 in range(8):
                    cc = half * 8 + c8
                    nc.tensor.transpose(
                        tps[:, c8, :], p[:, cc * P : (cc + 1) * P], identb[:]
                    )
                if half == 0:
                    nc.vector.tensor_copy(pt[:, 0:8, :], tps[:])
                else:
                    nc.scalar.copy(pt[:, 8:16, :], tps[:])

            ops = psO.tile([P, 130], f32, tag="psO")
            for c in range(NB):
                nc.tensor.matmul(
                    ops[:, 0:129],
                    pt[:, c, :],
                    rhs_att[:, c, :],
                    start=(c == 0),
                    stop=(c == NB - 1),
                )

            # epilogue: u = num / den ; out = elu(u) = relu(u) + exp(min(u,0)) - 1
            rs = spool.tile([P, 1], f32, tag="rs")
            nc.vector.reciprocal(rs[:], ops[:, 128:129])
            m0 = spool.tile([P, O], f32, tag="m0")
            nc.vector.tensor_scalar(m0[:], ops[:, 0:128], rs[:], 0.0, Alu.mult, Alu.min)
            e2 = spool.tile([P, O], f32, tag="e2")
            nc.scalar.activation(e2[:], m0[:], Act.Exp)
            v0 = spool.tile([P, O], f32, tag="v0")
            nc.vector.tensor_scalar(v0[:], ops[:, 0:128], rs[:], 0.0, Alu.mult, Alu.max)
            nc.vector.scalar_tensor_tensor(
                outbuf[:, rb, :], e2[:], -1.0, v0[:], Alu.add, Alu.add
            )

        nc.sync.dma_start(out_d.rearrange("(rb p) o -> p rb o", p=P), outbuf[:])

    nc.finalize()
    return nc


def get_nc():
    if "nc" not in _CACHE:
        _CACHE["nc"] = _build_nc()
    return _CACHE["nc"]


def run(h, j, adj, W1, W2, a, trace=False):
    from concourse.bass_utils import run_bass_kernel_spmd

    nc = get_nc()
    in_maps = [
        {
            "h": np.ascontiguousarray(h[b]),
            "j": np.ascontiguousarray(j[b]),
            "adj": np.ascontiguousarray(adj[b]),
            "W1": np.ascontiguousarray(W1),
            "W2": np.ascontiguousarray(W2),
            "a": np.ascontiguousarray(a),
        }
        for b in range(B)
    ]
    res = run_bass_kernel_spmd(nc, in_maps, core_ids=list(range(B)), trace=trace)
    out = np.stack([res.results[b]["out"] for b in range(B)], axis=0)
    return out, res


def kernel(h, j, adj, W1, W2, a):
    h = np.asarray(h, dtype=np.float32)
    j = np.asarray(j, dtype=np.float32)
    adj = np.asarray(adj, dtype=np.int32)
    W1 = np.asarray(W1, dtype=np.float32)
    W2 = np.asarray(W2, dtype=np.float32)
    a = np.asarray(a, dtype=np.float32)
    out, _ = run(h, j, adj, W1, W2, a, trace=False)
    return out


# revision 13
# speedup vs baseline: 2.7973x; 2.7973x over previous
"""GAT layer (nn_SPACY_GraphAttentionLayer) Trainium2 Bass kernel.

Data-parallel over batch: 8 graphs -> 8 NeuronCores, one graph per core.

Math (per graph):
  Wh = h @ W1, Wj = j @ W2, V = Wh + Wj
  Wh1_i = (Wh @ a1)_i,  Wh2_k = (Wh @ a2)_k
  z[i,k] = Wh1_i + Wh2_k
  logits = where(adj>0, leaky_relu(z, 0.2), -inf)
  A = softmax(logits, axis k);  out = elu(A @ V)

Factorization used on-chip:
  exp(lrelu(z)) = exp(0.8*relu(z)) * exp(0.2*Wh1_i) * exp(0.2*Wh2_k)
  The exp(0.2*Wh1_i) row factor cancels in the softmax.  With
  q = exp(0.8*relu(z)), p = adj * q, e_k = exp(0.2*Wh2_k):
     A[i,k] = p[i,k]*e_k / sum_k p[i,k]*e_k
     out_row = elu( (p_row @ (e*V)) / (p_row @ e) )
"""
import sys
import numpy as np

sys.path.insert(0, "/opt/trn_rl_repo")

B, N, F, O = 8, 2048, 256, 128
P = 128
NB = N // P  # 16 row/col chunks

_CACHE = {}
VARIANT = {"adj_hwdge": False, "skip_mask": False, "deep_bufs": True, "skip_exp": False, "skip_tp": False, "skip_mm": False, "one_adj": False, "adj4": False}


def _build_nc(repeat=1, loop_iters=1):
    import ml_dtypes
    from contextlib import ExitStack
    import concourse.bass as bass
    import concourse.tile as tile
    from concourse import bacc, mybir

    f32 = mybir.dt.float32
    bf16 = mybir.dt.bfloat16
    fp16 = mybir.dt.float16
    i32 = mybir.dt.int32
    Alu = mybir.AluOpType
    Act = mybir.ActivationFunctionType

    nc = bacc.Bacc()
    h_d = nc.dram_tensor("h", [N, F], f32, kind="ExternalInput")
    j_d = nc.dram_tensor("j", [N, F], f32, kind="ExternalInput")
    adj_d = nc.dram_tensor("adj", [N, N], i32, kind="ExternalInput")
    W1_d = nc.dram_tensor("W1", [F, O], f32, kind="ExternalInput")
    W2_d = nc.dram_tensor("W2", [F, O], f32, kind="ExternalInput")
    a_d = nc.dram_tensor("a", [2 * O, 1], f32, kind="ExternalInput")
    out_d = nc.dram_tensor("out", [N, O], f32, kind="ExternalOutput")

    identb_d = nc.inline_tensor(np.eye(P, dtype=ml_dtypes.bfloat16), name="identb")
    identh_d = nc.inline_tensor(np.eye(P, dtype=np.float16), name="identh")
    identf_d = nc.inline_tensor(np.eye(P, dtype=np.float32), name="identf")

    with tile.TileContext(nc) as tc, ExitStack() as ctx:
        cpool = ctx.enter_context(tc.tile_pool(name="cpool", bufs=1))
        wpool = ctx.enter_context(tc.tile_pool(name="wpool", bufs=1))
        bpool = ctx.enter_context(tc.tile_pool(name="bpool", bufs=1))
        apool = ctx.enter_context(tc.tile_pool(name="apool", bufs=(3 if VARIANT.get("adj4") else 6) if VARIANT.get("deep_bufs") else 4))
        qpool = ctx.enter_context(tc.tile_pool(name="qpool", bufs=3 if VARIANT.get("deep_bufs") else 2))
        spool = ctx.enter_context(tc.tile_pool(name="spool", bufs=2))
        db = VARIANT.get("deep_bufs", False)
        psM = ctx.enter_context(tc.tile_pool(name="psM", bufs=1, space="PSUM"))
        psT = ctx.enter_context(tc.tile_pool(name="psT", bufs=4 if db else 3, space="PSUM"))
        psO = ctx.enter_context(tc.tile_pool(name="psO", bufs=3 if db else 2, space="PSUM"))

        # ---------------- Stage A: weights prep ----------------
        identb = cpool.tile([P, P], bf16, tag="identb")
        nc.sync.dma_start(identb[:], identb_d[:])
        identf = cpool.tile([P, P], f32, tag="identf")
        nc.sync.dma_start(identf[:], identf_d[:])
        identh = cpool.tile([P, P], fp16, tag="identh")
        nc.sync.dma_start(identh[:], identh_d[:])
        ones1 = cpool.tile([1, P], f32, tag="ones1")
        nc.vector.memset(ones1[:], 1.0)

        w1s = wpool.tile([P, 2, O], f32, tag="w1s")
        nc.sync.dma_start(w1s[:], W1_d.rearrange("(c p) o -> p c o", p=P))
        w2s = wpool.tile([P, 2, O], f32, tag="w2s")
        nc.sync.dma_start(w2s[:], W2_d.rearrange("(c p) o -> p c o", p=P))
        a12 = wpool.tile([P, 2], f32, tag="a12")
        nc.sync.dma_start(a12[:], a_d.rearrange("(c p) one -> p (c one)", p=P))

        w1b = wpool.tile([P, 2, O], fp16, tag="w1b")
        nc.vector.tensor_copy(w1b[:], w1s[:])
        w2b = wpool.tile([P, 2, O], fp16, tag="w2b")
        nc.vector.tensor_copy(w2b[:], w2s[:])
        a12b = wpool.tile([P, 2], fp16, tag="a12b")
        nc.vector.tensor_copy(a12b[:], a12[:])

        # W1^T chunks (bf16) then w1a | w2a = W1 @ [a1 | a2]
        w1t_ps = psM.tile([P, 2, P], fp16, tag="psM")
        for c in range(2):
            nc.tensor.transpose(w1t_ps[:, c, :], w1b[:, c, :], identh[:])
        w1t = wpool.tile([P, 2, P], fp16, tag="w1t")
        nc.vector.tensor_copy(w1t[:], w1t_ps[:])
        wab_ps = psM.tile([P, 2, 2], f32, tag="psM")
        for c in range(2):
            nc.tensor.matmul(wab_ps[:, c, :], w1t[:, c, :], a12b[:], start=True, stop=True)
        wab = wpool.tile([P, 2, 2], fp16, tag="wab")
        nc.vector.tensor_copy(wab[:], wab_ps[:])

        # rhs for projections: r1 = [W1 | w1a | w2a], r2z = [W2 | 0 | 0]
        r1 = wpool.tile([P, 2, 130], fp16, tag="r1")
        nc.vector.tensor_copy(r1[:, :, 0:128], w1b[:])
        nc.vector.tensor_copy(r1[:, :, 128:130], wab[:])
        r2z = wpool.tile([P, 2, 130], fp16, tag="r2z")
        nc.vector.memset(r2z[:], 0.0)
        nc.vector.tensor_copy(r2z[:, :, 0:128], w2b[:])

        # ---------------- Stage B: load h/j, transpose, project ----------------
        # (repeat/loop_iters replay stages B+C for on-device timing)
        def body():
            for _rep in range(repeat):
                stage_bc(nc, tc, mybir, Alu, Act,
                         cpool, wpool, bpool, apool, qpool, spool, psM, psT, psO,
                         h_d, j_d, adj_d, out_d,
                         identb, identf, identh, ones1, r1, r2z, w2b)
        if loop_iters > 1:
            ET = mybir.EngineType
            with tc.For_i(0, loop_iters, 1,
                          hint_engines=(ET.PE, ET.DVE, ET.Activation, ET.SP)):
                body()
        else:
            body()

    nc.finalize()
    return nc


def stage_bc(nc, tc, mybir, Alu, Act,
             cpool, wpool, bpool, apool, qpool, spool, psM, psT, psO,
             h_d, j_d, adj_d, out_d,
             identb, identf, identh, ones1, r1, r2z, w2b):
    import ml_dtypes
    f32 = mybir.dt.float32
    bf16 = mybir.dt.bfloat16
    fp16 = mybir.dt.float16
    if True:
        hf = bpool.tile([P, NB, F], f32, tag="hf")
        nc.sync.dma_start(hf[:], h_d.rearrange("(n p) f -> p n f", p=P))
        jf = bpool.tile([P, NB, F], f32, tag="jf")
        nc.sync.dma_start(jf[:], j_d.rearrange("(n p) f -> p n f", p=P))
        hb = bpool.tile([P, NB, F], fp16, tag="hb")
        nc.vector.tensor_copy(hb[:], hf[:])
        jb = bpool.tile([P, NB, F], fp16, tag="jb")
        nc.scalar.copy(jb[:], jf[:])

        hT0 = bpool.tile([P, NB, P], fp16, tag="hT0")
        hT1 = bpool.tile([P, NB, P], fp16, tag="hT1")
        jT0 = bpool.tile([P, NB, P], fp16, tag="jT0")
        jT1 = bpool.tile([P, NB, P], fp16, tag="jT1")
        for g in range(4):  # groups of 4 chunks
            tph = psT.tile([P, 8, P], fp16, tag="psT")
            tpj = psT.tile([P, 8, P], fp16, tag="psT")
            for i in range(4):
                c16 = 4 * g + i
                nc.tensor.transpose(tph[:, i, :], hb[:, c16, 0:128], identh[:])
                nc.tensor.transpose(tph[:, 4 + i, :], hb[:, c16, 128:256], identh[:])
                nc.tensor.transpose(tpj[:, i, :], jb[:, c16, 0:128], identh[:])
                nc.tensor.transpose(tpj[:, 4 + i, :], jb[:, c16, 128:256], identh[:])
            s = slice(4 * g, 4 * g + 4)
            nc.vector.tensor_copy(hT0[:, s, :], tph[:, 0:4, :])
            nc.vector.tensor_copy(hT1[:, s, :], tph[:, 4:8, :])
            nc.scalar.copy(jT0[:, s, :], tpj[:, 0:4, :])
            nc.scalar.copy(jT1[:, s, :], tpj[:, 4:8, :])

        wh1 = bpool.tile([P, NB], f32, tag="wh1")    # Wh1 column per row-block
        wh2g = bpool.tile([P, NB], f32, tag="wh2g")  # Wh2 column per col-chunk
        ecf = bpool.tile([P, NB], f32, tag="ecf")    # e = exp(0.2*Wh2) per chunk
        rhs_att = bpool.tile([P, NB, 129], fp16, tag="rhs_att")  # [e*V | e] per chunk

        for c16 in range(NB):
            psv = psO.tile([P, 130], f32, tag="psO")
            nc.tensor.matmul(psv[:], hT0[:, c16, :], r1[:, 0, :], start=True, stop=False)
            nc.tensor.matmul(psv[:], hT1[:, c16, :], r1[:, 1, :], start=False, stop=False)
            nc.tensor.matmul(psv[:], jT0[:, c16, :], r2z[:, 0, :], start=False, stop=False)
            nc.tensor.matmul(psv[:], jT1[:, c16, :], r2z[:, 1, :], start=False, stop=True)
            nc.vector.tensor_copy(wh1[:, c16 : c16 + 1], psv[:, 128:129])
            nc.vector.tensor_copy(wh2g[:, c16 : c16 + 1], psv[:, 129:130])
            nc.scalar.activation(ecf[:, c16 : c16 + 1], psv[:, 129:130], Act.Exp, scale=0.2)
            # V' = e_k * (Wh + Wj), bf16; plus e column at position 128
            nc.vector.tensor_scalar(
                rhs_att[:, c16, 0:128], psv[:, 0:128], ecf[:, c16 : c16 + 1], None, Alu.mult
            )
            nc.vector.tensor_copy(rhs_att[:, c16, 128:129], ecf[:, c16 : c16 + 1])

        # Wh2 broadcast tile [P, N] bf16: wh2g -> transpose -> flatten row -> ones outer-product
        t16ps = psM.tile([16, P], f32, tag="psM")
        nc.tensor.transpose(t16ps[:], wh2g[:], identf[:])
        t16s = bpool.tile([16, P], f32, tag="t16s")
        nc.vector.tensor_copy(t16s[:], t16ps[:])
        wh2row = bpool.tile([1, N], f32, tag="wh2row")
        nc.sync.dma_start(wh2row[:], t16s[:])  # SBUF->SBUF partition-major flatten
        wh2bc = bpool.tile([P, N], fp16, tag="wh2bc")
        for s4 in range(4):
            psbc = psM.tile([P, 512], f32, tag="psM")
            nc.tensor.matmul(
                psbc[:], ones1[:], wh2row[0:1, s4 * 512 : (s4 + 1) * 512], start=True, stop=True
            )
            nc.vector.tensor_copy(wh2bc[:, s4 * 512 : (s4 + 1) * 512], psbc[:])

        # ---------------- Stage C: attention row-blocks ----------------
        outbuf = bpool.tile([P, NB, O], f32, tag="outbuf")
        _ptkeep = [None]
        _adjkeep = [None]
        adj4_tiles = {}
        if VARIANT.get("adj4"):
            adj_r = adj_d.rearrange("(g p) k -> p g k", p=P)  # g = row-block
        for rb in range(NB):
            if VARIANT.get("adj4"):
                g4 = rb // 2
                if rb % 2 == 0:
                    a4 = apool.tile([P, 2, N], bf16, tag="adjb")
                    nc.gpsimd.dma_start(a4[:], adj_r[:, g4 * 2 : (g4 + 1) * 2, :])
                    adj4_tiles[g4] = a4
                adjb = adj4_tiles[g4][:, rb % 2, :]
            elif VARIANT.get("one_adj"):
                if rb == 0:
                    adjb = apool.tile([P, N], bf16, tag="adjb")
                    nc.gpsimd.dma_start(adjb[:], adj_d[rb * P : (rb + 1) * P, :])
                    _adjkeep[0] = adjb
                else:
                    adjb = _adjkeep[0]
            elif VARIANT.get("adj_hwdge"):
                adji = apool.tile([P, N], mybir.dt.int32, tag="adjb")
                nc.sync.dma_start(adji[:], adj_d[rb * P : (rb + 1) * P, :])
                adjb = adji.bitcast(mybir.dt.uint16).rearrange(
                    "p (k two) -> p k two", two=2)[:, :, 0]
            else:
                adjb = apool.tile([P, N], bf16, tag="adjb")
                nc.gpsimd.dma_start(adjb[:], adj_d[rb * P : (rb + 1) * P, :])  # i32->bf16 cast

            r = qpool.tile([P, N], fp16, tag="r")
            nc.vector.tensor_scalar(
                r[:], wh2bc[:], wh1[:, rb : rb + 1], 0.0, Alu.add, Alu.max
            )
            q = qpool.tile([P, N], bf16, tag="q")
            if VARIANT.get("skip_exp"):
                nc.vector.tensor_copy(q[:], r[:])
            else:
                nc.scalar.activation(q[:], r[:], Act.Exp, scale=0.8)
            if VARIANT.get("skip_mask"):
                p = q
            else:
                p = qpool.tile([P, N], bf16, tag="p")
                nc.vector.tensor_tensor(p[:], q[:], adjb[:], Alu.mult)

            if VARIANT.get("skip_tp"):
                if rb == 0:
                    pt = qpool.tile([P, NB, P], bf16, tag="pt")
                    nc.vector.memset(pt[:], 0.001)
                    _ptkeep[0] = pt
                else:
                    pt = _ptkeep[0]
            else:
                pt = qpool.tile([P, NB, P], bf16, tag="pt")
            for half in range(2 if not VARIANT.get("skip_tp") else 0):
                tps = psT.tile([P, 8, P], bf16, tag="psT")
                for c8 in range(8):
                    cc = half * 8 + c8
                    nc.tensor.transpose(
                        tps[:, c8, :], p[:, cc * P : (cc + 1) * P], identb[:]
                    )
                if half == 0:
                    nc.vector.tensor_copy(pt[:, 0:8, :], tps[:])
                else:
                    nc.scalar.copy(pt[:, 8:16, :], tps[:])

            ops = psO.tile([P, 130], f32, tag="psO")
            nmm = 1 if VARIANT.get("skip_mm") else NB
            for c in range(nmm):
                nc.tensor.matmul(
                    ops[:, 0:129],
                    pt[:, c, :],
                    rhs_att[:, c, :],
                    start=(c == 0),
                    stop=(c == nmm - 1),
                )

            # epilogue: u = num / den ; out = elu(u) = relu(u) + exp(min(u,0)) - 1
            rs = spool.tile([P, 1], f32, tag="rs")
            nc.vector.reciprocal(rs[:], ops[:, 128:129])
            m0 = spool.tile([P, O], f32, tag="m0")
            nc.vector.tensor_scalar(m0[:], ops[:, 0:128], rs[:], 0.0, Alu.mult, Alu.min)
            e2 = spool.tile([P, O], f32, tag="e2")
            nc.scalar.activation(e2[:], m0[:], Act.Exp)
            v0 = spool.tile([P, O], f32, tag="v0")
            nc.vector.tensor_scalar(v0[:], ops[:, 0:128], rs[:], 0.0, Alu.mult, Alu.max)
            nc.vector.scalar_tensor_tensor(
                outbuf[:, rb, :], e2[:], -1.0, v0[:], Alu.add, Alu.add
            )

        nc.sync.dma_start(out_d.rearrange("(rb p) o -> p rb o", p=P), outbuf[:])


def get_nc(repeat=1, loop_iters=1):
    key = ("nc", repeat, loop_iters, tuple(sorted(VARIANT.items())))
    if key not in _CACHE:
        _CACHE[key] = _build_nc(repeat, loop_iters)
    return _CACHE[key]


def run(h, j, adj, W1, W2, a, trace=False):
    from concourse.bass_utils import run_bass_kernel_spmd

    nc = get_nc()
    in_maps = [
        {
            "h": np.ascontiguousarray(h[b]),
            "j": np.ascontiguousarray(j[b]),
            "adj": np.ascontiguousarray(adj[b]),
            "W1": np.ascontiguousarray(W1),
            "W2": np.ascontiguousarray(W2),
            "a": np.ascontiguousarray(a),
        }
        for b in range(B)
    ]
    res = run_bass_kernel_spmd(nc, in_maps, core_ids=list(range(B)), trace=trace)
    out = np.stack([res.results[b]["out"] for b in range(B)], axis=0)
    return out, res


def kernel(h, j, adj, W1, W2, a):
    h = np.asarray(h, dtype=np.float32)
    j = np.asarray(j, dtype=np.float32)
    adj = np.asarray(adj, dtype=np.int32)
    W1 = np.asarray(W1, dtype=np.float32)
    W2 = np.asarray(W2, dtype=np.float32)
    a = np.asarray(a, dtype=np.float32)
    out, _ = run(h, j, adj, W1, W2, a, trace=False)
    return out
